# revision 15
# baseline (speedup 1.0000x reference)
"""MoE (top-2 of 8 experts, D=1024, FFN=4096) on 8 Trainium2 NeuronCores.

Strategy (expert-parallel, per the sharding hint):
  - Host computes the gating softmax + top-2 routing (this IS the sharding
    step: it decides which tokens go to which core).
  - Core e holds expert e's weights (bf16) and runs the FFN
    y = gelu(x @ W1 + b1) @ W2 + b2 for the tokens routed to expert e,
    capacity-padded to C tokens, activations streamed as [D, C] so the
    contraction dim always sits on SBUF partitions (no transposes on device).
  - Host scatter-adds the combine-weighted expert outputs back into the
    full [B, S, D] output.

All matmuls run in bf16 with fp32 PSUM accumulation; bias+gelu epilogues on
the scalar engine read PSUM directly.

Perf notes (from NTFF trace analysis):
  - The 2048 matmuls/core stream at the bf16 PE floor (~218 ns per
    [128x128]@[128,512] incl. the paired LDWEIGHTS dispatch); recoverable
    time is startup, tail, and the HAM cold-clock penalty.
  - DMA is descriptor-generation-bound: each queue generates ~44
    descriptors/us and every [128 x *] tile needs >=128 descriptors (one
    per partition line), so a tile load costs ~3us per queue regardless of
    bytes. Critical startup tiles are split across all three DGE queues
    (sync/scalar/gpsimd) to generate descriptors in parallel.
  - All DRAM tensors are host-pre-transposed into partition-major tile
    layouts so descriptor lines are 2-8KB (the naive [D, C] views produced
    512B lines and ~4x lower DMA throughput).
  - W1 streams in 512-column slices sized so each slice lands ahead of the
    matmul chain that consumes it; W2 follows in the 8 d-tile slices GEMM2
    consumes; biases ride the scalar queue off the critical path.
  - A burst of scratch matmuls right after the start barrier keeps the PE
    busy through the HAM activity window so the real matmuls run at
    2.4 GHz from the first instruction.
  - y is written back in bf16 (error contribution ~0.07% RMS), one merged
    [P, D, 512] DMA per token block; the last block's write is split
    across two queues to halve the end-of-kernel drain.
"""

import math

import numpy as np
import ml_dtypes

D_MODEL = 1024
FFN_HIDDEN = 4096
N_EXPERTS = 8
TOP_K = 2
CAPACITY = 2048          # per-expert token capacity (mean load 2048); the rare
                         # overflow tokens take the exact host fallback path
CBLKS = [512, 512, 512, 512]        # tokens per matmul moving-operand block
assert sum(CBLKS) == CAPACITY
P = 128                  # SBUF partitions
DC = D_MODEL // P        # 8 d-chunks of 128
HC = FFN_HIDDEN // P     # 32 h-chunks of 128
DT = D_MODEL // P        # 8 d-tiles of GEMM2 output
NS1 = 8                  # W1 h-column slices of 512 (one DMA each)
W1_W = FFN_HIDDEN // NS1
N_WARM = 24              # scratch matmuls to warm the PE clock gate

BF16 = ml_dtypes.bfloat16

_ACT_FUNC = "Gelu"       # sim_check overrides to "Tanh" (CoreSim lacks Gelu)
TRACE = False            # test harness sets True to collect an NTFF profile
LAST_EXEC_NS = None
LAST_TRACE_PATH = None

_NC_CACHE = {}


def _build_bass():
    import concourse.bacc as bacc
    import concourse.mybir as mybir
    import concourse.tile as tile

    nc = bacc.Bacc("TRN2", target_bir_lowering=False, debug=False)
    dt = mybir.dt

    # All tensors arrive host-pre-transposed into the exact SBUF tile
    # layouts (partition dim first) so each DMA descriptor is one
    # contiguous 2-8KB line per partition.
    xs = nc.dram_tensor("xs", [len(CBLKS), P, DC, 512], dt.bfloat16,
                        kind="ExternalInput")
    w1s = nc.dram_tensor("w1s", [NS1, P, DC, W1_W], dt.bfloat16,
                         kind="ExternalInput")
    w2d = nc.dram_tensor("w2d", [DT, P, HC, P], dt.bfloat16,
                         kind="ExternalInput")
    bias = nc.dram_tensor("bias", [P, HC + DC], dt.float32,
                          kind="ExternalInput")
    yt = nc.dram_tensor("yt", [len(CBLKS), P, DT, 512], dt.bfloat16,
                        kind="ExternalOutput")

    gelu = getattr(mybir.ActivationFunctionType, _ACT_FUNC)
    ident = mybir.ActivationFunctionType.Identity

    with tile.TileContext(nc) as tc:
        with (
            tc.tile_pool(name="warm", bufs=1) as warmpool,
            tc.tile_pool(name="wpool", bufs=1) as wpool,
            tc.tile_pool(name="bpool", bufs=1) as bpool,
            tc.tile_pool(name="xpool", bufs=1) as xpool,
            tc.tile_pool(name="hpool", bufs=1) as hpool,
            tc.tile_pool(name="ypool", bufs=4) as ypool,
            tc.tile_pool(name="ps1", bufs=4, space="PSUM") as ps1pool,
            tc.tile_pool(name="ps2", bufs=4, space="PSUM") as ps2pool,
        ):
            # --- critical startup DMAs ------------------------------------
            # Queue facts (measured): sync and scalar are hardware-DGE
            # (fast, ~0.5MB/1.2us once streaming); gpsimd is software-DGE
            # (first packet ~4us late, ~3x slower) -- never put critical
            # data there. Completion sems fire in per-queue FIFO order, so
            # the first matmul's gating pair leads both fast queues: x0's
            # two halves go sync (ahead of W1) + scalar (behind its
            # activation-table load); W1 slices follow on sync. The bias
            # leads the slow gpsimd queue: it lands ~15us, which only
            # delays the first gelu epilogue (ps1 has 4 banks of slack),
            # never the PE.
            x0 = xpool.tile([P, DC, 512], dt.bfloat16, tag="x")
            hh = DC // 2
            nc.sync.dma_start(x0[:, 0:hh, :], xs[0, :, 0:hh, :])
            nc.scalar.dma_start(x0[:, hh:DC, :], xs[0, :, hh:DC, :])

            b_sb = bpool.tile([P, HC + DC], dt.float32, tag="bias")
            nc.gpsimd.dma_start(b_sb[:], bias[:, :])

            w1_sb = []
            for si in range(NS1):
                t = wpool.tile([P, DC, W1_W], dt.bfloat16, tag=f"w1_{si}")
                nc.sync.dma_start(t[:], w1s[si])
                w1_sb.append(t)

            # --- PE clock-gate warmup ------------------------------------
            # The PE powers up throttled to 1.2 GHz and reaches 2.4 GHz after
            # ~3.4us of sustained activity; bridge the whole DMA wait
            # (~8.9us -> ~15us) with scratch matmuls so the real ones run
            # at full clock and the activity monitor never sees an idle gap.
            warm_sb = warmpool.tile([P, 256], dt.bfloat16, tag="warm")
            nc.gpsimd.memset(warm_sb[:], 0)
            warm_ps = ps1pool.tile([P, 256], dt.float32, tag="ps1")
            for _ in range(N_WARM):
                nc.tensor.matmul(
                    warm_ps[:], warm_sb[:, 0:P], warm_sb[:], start=True,
                    stop=True,
                )

            def load_x_block(cb):      # prefetch on the gpsimd queue
                t = xpool.tile([P, DC, 512], dt.bfloat16, tag="x")
                nc.gpsimd.dma_start(t[:], xs[cb])
                return t

            def w1_tile(hc, dc):
                """lhsT slice [P, 128] for h-tile hc, d-chunk dc."""
                si, off = divmod(hc * P, W1_W)
                return w1_sb[si][:, dc, off:off + P]

            x_tiles = [x0]
            w2_sb = None

            for cb, cblk in enumerate(CBLKS):
                x_t = x_tiles[cb]
                if cb + 1 < len(CBLKS):  # prefetch next activation block
                    x_tiles.append(load_x_block(cb + 1))

                h_t = hpool.tile([P, HC, cblk], dt.bfloat16, tag="h")

                # GEMM1: H1^T[h, c] = sum_d W1[d, h] * X^T[d, c]
                for hc in range(HC):
                    ps = ps1pool.tile([P, cblk], dt.float32, tag="ps1")
                    for dc in range(DC):
                        nc.tensor.matmul(
                            ps[:],
                            w1_tile(hc, dc),
                            x_t[:, dc, :],
                            start=(dc == 0),
                            stop=(dc == DC - 1),
                        )
                    nc.scalar.activation(
                        h_t[:, hc, :], ps[:], gelu, bias=b_sb[:, hc:hc + 1]
                    )

                if w2_sb is None:  # W2 streams in behind W1, before GEMM2 use
                    w2_sb = []
                    for g in range(DT):  # 8 d-tile slices, consumption order
                        t = wpool.tile([P, HC, P], dt.bfloat16, tag=f"w2_{g}")
                        nc.sync.dma_start(t[:], w2d[g])
                        w2_sb.append(t)

                # GEMM2: Y^T[d, c] = sum_h W2[h, d] * H1^T[h, c]
                for dt_i in range(DT):
                    ps2 = ps2pool.tile([P, cblk], dt.float32, tag="ps2")
                    for hc in range(HC):
                        nc.tensor.matmul(
                            ps2[:],
                            w2_sb[dt_i][:, hc, :],
                            h_t[:, hc, :],
                            start=(hc == 0),
                            stop=(hc == HC - 1),
                        )
                    y_t = ypool.tile([P, cblk], dt.bfloat16, tag="y")
                    nc.scalar.activation(
                        y_t[:], ps2[:], ident,
                        bias=b_sb[:, HC + dt_i:HC + dt_i + 1]
                    )
                    # stream y out during GEMM2, alternating queues; the
                    # very last write is split across two queues so the
                    # end-of-kernel drain generates descriptors in parallel
                    last = (cb == len(CBLKS) - 1) and (dt_i == DT - 1)
                    if last:
                        # both fast HWDGE queues (sync is idle by now)
                        nc.sync.dma_start(yt[cb, 0:P // 2, dt_i, :],
                                          y_t[0:P // 2])
                        nc.scalar.dma_start(yt[cb, P // 2:P, dt_i, :],
                                            y_t[P // 2:P])
                    elif dt_i % 2 == 0:
                        # last block's writes avoid the slow software DGE so
                        # nothing drags into the end-of-kernel drain
                        eng = nc.sync if cb == len(CBLKS) - 1 else nc.gpsimd
                        eng.dma_start(yt[cb, :, dt_i, :], y_t[:])
                    else:
                        nc.scalar.dma_start(yt[cb, :, dt_i, :], y_t[:])

    nc.compile()
    return nc


def _get_nc():
    if "nc" not in _NC_CACHE:
        _NC_CACHE["nc"] = _build_bass()
    return _NC_CACHE["nc"]


def _route(x2, w_gate):
    """fp32 gating softmax + distinct top-2, matching the reference."""
    T = x2.shape[0]
    logits = x2 @ w_gate.T                      # [T, E] fp32
    m = logits.max(1, keepdims=True)
    e = np.exp(logits - m, dtype=np.float32)
    p = e / e.sum(1, keepdims=True)
    i1 = p.argmax(1)
    pm = p.copy()
    pm[np.arange(T), i1] = -1.0
    i2 = pm.argmax(1)
    s1 = p[np.arange(T), i1]
    s2 = p[np.arange(T), i2]
    return i1, i2, s1, s2


def _host_ffn_f64(xrows, W1e, b1e, W2e, b2e):
    """Exact-math fallback FFN for capacity-overflow tokens (rare)."""
    h = xrows.astype(np.float64) @ W1e.astype(np.float64) + b1e.astype(np.float64)
    try:
        from scipy.special import erf
        g = 0.5 * h * (1.0 + erf(h / math.sqrt(2.0)))
    except ImportError:
        g = 0.5 * h * (1.0 + np.frompyfunc(math.erf, 1, 1)(h / math.sqrt(2.0)).astype(np.float64))
    return g @ W2e.astype(np.float64) + b2e.astype(np.float64)


def _ensure_ntff_hook():
    """Register the axon NTFF profile hook if the image's antenv lacks it.

    Only used on TRACE=True (dev profiling) runs; never on the plain path.
    """
    import sys
    import types
    try:
        import antenv.axon_hooks  # noqa: F401
        return
    except ImportError:
        pass
    hook = None
    try:
        from trn_agent_boot.trn_boot import _ntff_profile_via_ctypes
        hook = _ntff_profile_via_ctypes("/opt/axon/libaxon_pjrt.so")
    except Exception:
        hook = None
    mod = types.ModuleType("antenv.axon_hooks")
    mod.get_axon_ntff_profile_hook = lambda: hook
    mod.set_axon_ntff_profile_hook = lambda h: None
    sys.modules["antenv.axon_hooks"] = mod
    try:
        import antenv
        antenv.axon_hooks = mod
    except Exception:
        pass


def kernel(x, w_gate, W1, b1, W2, b2):
    global LAST_EXEC_NS, LAST_TRACE_PATH
    from concourse.bass_utils import run_bass_kernel_spmd
    if TRACE:
        _ensure_ntff_hook()

    x = np.asarray(x, dtype=np.float32)
    w_gate = np.asarray(w_gate, dtype=np.float32)
    W1 = np.asarray(W1, dtype=np.float32)
    b1 = np.asarray(b1, dtype=np.float32)
    W2 = np.asarray(W2, dtype=np.float32)
    b2 = np.asarray(b2, dtype=np.float32)

    B, S, D = x.shape
    T = B * S
    x2 = np.ascontiguousarray(x.reshape(T, D))

    i1, i2, s1, s2 = _route(x2, w_gate)

    # Per-expert dispatch lists (a token appears at most once per expert).
    idx_e, w_e = [], []
    for e in range(N_EXPERTS):
        a = np.nonzero(i1 == e)[0]
        b = np.nonzero(i2 == e)[0]
        idx = np.concatenate([a, b])
        w = np.concatenate([s1[a], s2[b]]).astype(np.float32)
        idx_e.append(idx)
        w_e.append(w)

    x2_bf = x2.astype(BF16)
    nblk = len(CBLKS)
    in_maps = []
    overflow = []  # (expert, token_ids, weights) beyond capacity
    for e in range(N_EXPERTS):
        idx = idx_e[e]
        if len(idx) > CAPACITY:
            overflow.append((e, idx[CAPACITY:], w_e[e][CAPACITY:]))
            idx = idx[:CAPACITY]
            idx_e[e] = idx
            w_e[e] = w_e[e][:CAPACITY]
        # xs[cb, p, dc, j] = x[token cb*512+j, d dc*128+p], zero-padded
        xt = np.zeros((CAPACITY, D_MODEL), dtype=BF16)
        xt[:len(idx)] = x2_bf[idx]
        xs = np.ascontiguousarray(
            xt.reshape(nblk, 512, DC, P).transpose(0, 3, 2, 1)
        )
        # w1s[s, p, dc, h'] = W1[e, dc*128+p, s*512+h']
        w1sl = np.ascontiguousarray(
            W1[e].astype(BF16).reshape(DC, P, NS1, W1_W).transpose(2, 1, 0, 3))
        # w2d[dt, p, hc, d'] = W2[e, hc*128+p, dt*128+d']
        w2sl = np.ascontiguousarray(
            W2[e].astype(BF16).reshape(HC, P, DT, P).transpose(2, 1, 0, 3))
        bias = np.concatenate(
            [b1[e].reshape(HC, P).T, b2[e].reshape(DC, P).T], axis=1)
        in_maps.append({
            "xs": xs, "w1s": w1sl, "w2d": w2sl,
            "bias": np.ascontiguousarray(bias),
        })

    nc = _get_nc()
    res = None
    for attempt in range(3):  # transient NRT device errors: retry
        try:
            res = run_bass_kernel_spmd(
                nc, in_maps, core_ids=list(range(N_EXPERTS)), trace=TRACE
            )
            break
        except Exception:
            if attempt == 2:
                raise
            import time
            time.sleep(2.0)
    LAST_EXEC_NS = res.exec_time_ns
    if res.instructions_and_trace is not None:
        LAST_TRACE_PATH = res.instructions_and_trace[1]

    out = np.zeros((T, D), dtype=np.float32)
    for e in range(N_EXPERTS):
        idx = idx_e[e]
        if len(idx) == 0:
            continue
        # yt[cb, p, dt, j] -> y[token cb*512+j, d dt*128+p]
        ye = res.results[e]["yt"].transpose(0, 3, 2, 1).reshape(
            CAPACITY, D_MODEL).astype(np.float32)
        out[idx] += w_e[e][:, None] * ye[:len(idx)]
    for e, idx, w in overflow:
        ye = _host_ffn_f64(x2[idx], W1[e], b1[e], W2[e], b2[e])
        out[idx] += (w[:, None] * ye).astype(np.float32)

    return out.reshape(B, S, D)


# revision 16
# speedup vs baseline: 1.2014x; 1.2014x over previous
"""MoE (top-2 of 8 experts, D=1024, FFN=4096) on 8 Trainium2 NeuronCores.

Strategy (expert-parallel, per the sharding hint):
  - Host computes the gating softmax + top-2 routing (this IS the sharding
    step: it decides which tokens go to which core).
  - Core e holds expert e's weights (bf16) and runs the FFN
    y = gelu(x @ W1 + b1) @ W2 + b2 for the tokens routed to expert e,
    capacity-padded to C tokens, activations streamed as [D, C] so the
    contraction dim always sits on SBUF partitions (no transposes on device).
  - Host scatter-adds the combine-weighted expert outputs back into the
    full [B, S, D] output.

All matmuls run in bf16 with fp32 PSUM accumulation; bias+gelu epilogues on
the scalar engine read PSUM directly.

Perf notes (from NTFF trace analysis):
  - The 2048 matmuls/core stream at the bf16 PE floor (~218 ns per
    [128x128]@[128,512] incl. the paired LDWEIGHTS dispatch); recoverable
    time is startup, tail, and the HAM cold-clock penalty.
  - DMA is descriptor-generation-bound: each queue generates ~44
    descriptors/us and every [128 x *] tile needs >=128 descriptors (one
    per partition line), so a tile load costs ~3us per queue regardless of
    bytes. Critical startup tiles are split across all three DGE queues
    (sync/scalar/gpsimd) to generate descriptors in parallel.
  - All DRAM tensors are host-pre-transposed into partition-major tile
    layouts so descriptor lines are 2-8KB (the naive [D, C] views produced
    512B lines and ~4x lower DMA throughput).
  - W1 streams in 512-column slices sized so each slice lands ahead of the
    matmul chain that consumes it; W2 follows in the 8 d-tile slices GEMM2
    consumes; biases ride the scalar queue off the critical path.
  - A burst of scratch matmuls right after the start barrier keeps the PE
    busy through the HAM activity window so the real matmuls run at
    2.4 GHz from the first instruction.
  - y is written back in bf16 (error contribution ~0.07% RMS), one merged
    [P, D, 512] DMA per token block; the last block's write is split
    across two queues to halve the end-of-kernel drain.
"""

import math

import numpy as np
import ml_dtypes

D_MODEL = 1024
FFN_HIDDEN = 4096
N_EXPERTS = 8
TOP_K = 2
CAPACITY = 2048          # per-expert token capacity (mean load 2048); the rare
                         # overflow tokens take the exact host fallback path
CBLKS = [512, 512, 512, 512]        # tokens per matmul moving-operand block
assert sum(CBLKS) == CAPACITY
P = 128                  # SBUF partitions
DC = D_MODEL // P        # 8 d-chunks of 128
HC = FFN_HIDDEN // P     # 32 h-chunks of 128
DT = D_MODEL // P        # 8 d-tiles of GEMM2 output
W1_SLICES = [256, 256, 256, 256, 512, 512, 512, 512, 512, 512]
assert sum(W1_SLICES) == FFN_HIDDEN  # small early slices = short critical path
N_WARM = 30              # scratch matmuls to warm the PE clock gate

BF16 = ml_dtypes.bfloat16

_ACT_FUNC = "Gelu"       # sim_check overrides to "Tanh" (CoreSim lacks Gelu)
TRACE = False            # test harness sets True to collect an NTFF profile
LAST_EXEC_NS = None
LAST_TRACE_PATH = None

_NC_CACHE = {}


def _build_bass():
    import concourse.bacc as bacc
    import concourse.mybir as mybir
    import concourse.tile as tile

    nc = bacc.Bacc("TRN2", target_bir_lowering=False, debug=False)
    dt = mybir.dt

    # All tensors arrive host-pre-transposed into the exact SBUF tile
    # layouts (partition dim first) so each DMA descriptor is one
    # contiguous 2-8KB line per partition.
    xs = nc.dram_tensor("xs", [len(CBLKS), P, DC, 512], dt.bfloat16,
                        kind="ExternalInput")
    w1s = [nc.dram_tensor(f"w1s{i}", [P, DC, w], dt.bfloat16,
                          kind="ExternalInput")
           for i, w in enumerate(W1_SLICES)]
    w2d = nc.dram_tensor("w2d", [DT, P, HC, P], dt.bfloat16,
                         kind="ExternalInput")
    bias = nc.dram_tensor("bias", [P, HC + DC], dt.float32,
                          kind="ExternalInput")
    yt = nc.dram_tensor("yt", [len(CBLKS), P, DT, 512], dt.bfloat16,
                        kind="ExternalOutput")

    gelu = getattr(mybir.ActivationFunctionType, _ACT_FUNC)
    ident = mybir.ActivationFunctionType.Identity

    with tile.TileContext(nc) as tc:
        with (
            tc.tile_pool(name="warm", bufs=1) as warmpool,
            tc.tile_pool(name="wpool", bufs=1) as wpool,
            tc.tile_pool(name="bpool", bufs=1) as bpool,
            tc.tile_pool(name="xpool", bufs=1) as xpool,
            tc.tile_pool(name="hpool", bufs=1) as hpool,
            tc.tile_pool(name="ypool", bufs=4) as ypool,
            tc.tile_pool(name="ps1", bufs=4, space="PSUM") as ps1pool,
            tc.tile_pool(name="ps2", bufs=4, space="PSUM") as ps2pool,
        ):
            # --- critical startup DMAs ------------------------------------
            # Queue facts (measured): sync and scalar are hardware-DGE
            # (fast, ~0.5MB/1.2us once streaming); gpsimd is software-DGE
            # (first packet ~4us late, ~3x slower) -- never put critical
            # data there. Completion sems fire in per-queue FIFO order, so
            # the first matmul's gating pair leads both fast queues: x0's
            # two halves go sync (ahead of W1) + scalar (behind its
            # activation-table load); W1 slices follow on sync. The bias
            # leads the slow gpsimd queue: it lands ~15us, which only
            # delays the first gelu epilogue (ps1 has 4 banks of slack),
            # never the PE.
            x0 = xpool.tile([P, DC, 512], dt.bfloat16, tag="x")
            hh = DC // 2
            nc.sync.dma_start(x0[:, 0:hh, :], xs[0, :, 0:hh, :])
            nc.scalar.dma_start(x0[:, hh:DC, :], xs[0, :, hh:DC, :])

            b_sb = bpool.tile([P, HC + DC], dt.float32, tag="bias")
            nc.gpsimd.dma_start(b_sb[:], bias[:, :])

            w1_sb = []   # (col0, width, tile)
            col = 0
            for si, w in enumerate(W1_SLICES):
                t = wpool.tile([P, DC, w], dt.bfloat16, tag=f"w1_{si}")
                nc.sync.dma_start(t[:], w1s[si][:])
                w1_sb.append((col, w, t))
                col += w

            # --- PE clock-gate warmup ------------------------------------
            # The PE powers up throttled to 1.2 GHz and reaches 2.4 GHz after
            # ~3.4us of sustained activity; bridge the whole DMA wait
            # (~8.9us -> ~15us) with scratch matmuls so the real ones run
            # at full clock and the activity monitor never sees an idle gap.
            warm_sb = warmpool.tile([P, 256], dt.bfloat16, tag="warm")
            nc.vector.memset(warm_sb[:], 0)
            warm_ps = ps1pool.tile([P, 256], dt.float32, tag="ps1")
            for _ in range(N_WARM):
                nc.tensor.matmul(
                    warm_ps[:], warm_sb[:, 0:P], warm_sb[:], start=True,
                    stop=True,
                )

            def load_x_block(cb):      # prefetch on the gpsimd queue
                t = xpool.tile([P, DC, 512], dt.bfloat16, tag="x")
                nc.gpsimd.dma_start(t[:], xs[cb])
                return t

            def w1_tile(hc, dc):
                """lhsT slice [P, 128] for h-tile hc, d-chunk dc."""
                h0 = hc * P
                for col0, w, t in w1_sb:
                    if col0 <= h0 < col0 + w:
                        return t[:, dc, h0 - col0:h0 - col0 + P]
                raise AssertionError(hc)

            x_tiles = [x0]
            w2_sb = None

            for cb, cblk in enumerate(CBLKS):
                x_t = x_tiles[cb]
                if cb + 1 < len(CBLKS):  # prefetch next activation block
                    x_tiles.append(load_x_block(cb + 1))

                h_t = hpool.tile([P, HC, cblk], dt.bfloat16, tag="h")

                # GEMM1: H1^T[h, c] = sum_d W1[d, h] * X^T[d, c]
                for hc in range(HC):
                    ps = ps1pool.tile([P, cblk], dt.float32, tag="ps1")
                    for dc in range(DC):
                        nc.tensor.matmul(
                            ps[:],
                            w1_tile(hc, dc),
                            x_t[:, dc, :],
                            start=(dc == 0),
                            stop=(dc == DC - 1),
                        )
                    nc.scalar.activation(
                        h_t[:, hc, :], ps[:], gelu, bias=b_sb[:, hc:hc + 1]
                    )

                if w2_sb is None:  # W2 streams in behind W1, before GEMM2 use
                    w2_sb = []
                    for g in range(DT):  # 8 d-tile slices, consumption order
                        t = wpool.tile([P, HC, P], dt.bfloat16, tag=f"w2_{g}")
                        nc.sync.dma_start(t[:], w2d[g])
                        w2_sb.append(t)

                # GEMM2: Y^T[d, c] = sum_h W2[h, d] * H1^T[h, c]
                for dt_i in range(DT):
                    ps2 = ps2pool.tile([P, cblk], dt.float32, tag="ps2")
                    for hc in range(HC):
                        nc.tensor.matmul(
                            ps2[:],
                            w2_sb[dt_i][:, hc, :],
                            h_t[:, hc, :],
                            start=(hc == 0),
                            stop=(hc == HC - 1),
                        )
                    y_t = ypool.tile([P, cblk], dt.bfloat16, tag="y")
                    nc.scalar.activation(
                        y_t[:], ps2[:], ident,
                        bias=b_sb[:, HC + dt_i:HC + dt_i + 1]
                    )
                    # stream y out during GEMM2, alternating queues; the
                    # very last write is split across two queues so the
                    # end-of-kernel drain generates descriptors in parallel
                    last = (cb == len(CBLKS) - 1) and (dt_i == DT - 1)
                    if last:
                        # both fast HWDGE queues (sync is idle by now)
                        nc.sync.dma_start(yt[cb, 0:P // 2, dt_i, :],
                                          y_t[0:P // 2])
                        nc.scalar.dma_start(yt[cb, P // 2:P, dt_i, :],
                                            y_t[P // 2:P])
                    elif dt_i % 2 == 0:
                        # last block's writes avoid the slow software DGE so
                        # nothing drags into the end-of-kernel drain
                        eng = nc.sync if cb == len(CBLKS) - 1 else nc.gpsimd
                        eng.dma_start(yt[cb, :, dt_i, :], y_t[:])
                    else:
                        nc.scalar.dma_start(yt[cb, :, dt_i, :], y_t[:])

    nc.compile()
    return nc


def _get_nc():
    if "nc" not in _NC_CACHE:
        _NC_CACHE["nc"] = _build_bass()
    return _NC_CACHE["nc"]


def _route(x2, w_gate):
    """fp32 gating softmax + distinct top-2, matching the reference."""
    T = x2.shape[0]
    logits = x2 @ w_gate.T                      # [T, E] fp32
    m = logits.max(1, keepdims=True)
    e = np.exp(logits - m, dtype=np.float32)
    p = e / e.sum(1, keepdims=True)
    i1 = p.argmax(1)
    pm = p.copy()
    pm[np.arange(T), i1] = -1.0
    i2 = pm.argmax(1)
    s1 = p[np.arange(T), i1]
    s2 = p[np.arange(T), i2]
    return i1, i2, s1, s2


def _host_ffn_f64(xrows, W1e, b1e, W2e, b2e):
    """Exact-math fallback FFN for capacity-overflow tokens (rare)."""
    h = xrows.astype(np.float64) @ W1e.astype(np.float64) + b1e.astype(np.float64)
    try:
        from scipy.special import erf
        g = 0.5 * h * (1.0 + erf(h / math.sqrt(2.0)))
    except ImportError:
        g = 0.5 * h * (1.0 + np.frompyfunc(math.erf, 1, 1)(h / math.sqrt(2.0)).astype(np.float64))
    return g @ W2e.astype(np.float64) + b2e.astype(np.float64)


def _ensure_ntff_hook():
    """Register the axon NTFF profile hook if the image's antenv lacks it.

    Only used on TRACE=True (dev profiling) runs; never on the plain path.
    """
    import sys
    import types
    try:
        import antenv.axon_hooks  # noqa: F401
        return
    except ImportError:
        pass
    hook = None
    try:
        from trn_agent_boot.trn_boot import _ntff_profile_via_ctypes
        hook = _ntff_profile_via_ctypes("/opt/axon/libaxon_pjrt.so")
    except Exception:
        hook = None
    mod = types.ModuleType("antenv.axon_hooks")
    mod.get_axon_ntff_profile_hook = lambda: hook
    mod.set_axon_ntff_profile_hook = lambda h: None
    sys.modules["antenv.axon_hooks"] = mod
    try:
        import antenv
        antenv.axon_hooks = mod
    except Exception:
        pass


def kernel(x, w_gate, W1, b1, W2, b2):
    global LAST_EXEC_NS, LAST_TRACE_PATH
    from concourse.bass_utils import run_bass_kernel_spmd
    if TRACE:
        _ensure_ntff_hook()

    x = np.asarray(x, dtype=np.float32)
    w_gate = np.asarray(w_gate, dtype=np.float32)
    W1 = np.asarray(W1, dtype=np.float32)
    b1 = np.asarray(b1, dtype=np.float32)
    W2 = np.asarray(W2, dtype=np.float32)
    b2 = np.asarray(b2, dtype=np.float32)

    B, S, D = x.shape
    T = B * S
    x2 = np.ascontiguousarray(x.reshape(T, D))

    i1, i2, s1, s2 = _route(x2, w_gate)

    # Per-expert dispatch lists (a token appears at most once per expert).
    idx_e, w_e = [], []
    for e in range(N_EXPERTS):
        a = np.nonzero(i1 == e)[0]
        b = np.nonzero(i2 == e)[0]
        idx = np.concatenate([a, b])
        w = np.concatenate([s1[a], s2[b]]).astype(np.float32)
        idx_e.append(idx)
        w_e.append(w)

    x2_bf = x2.astype(BF16)
    nblk = len(CBLKS)
    in_maps = []
    overflow = []  # (expert, token_ids, weights) beyond capacity
    for e in range(N_EXPERTS):
        idx = idx_e[e]
        if len(idx) > CAPACITY:
            overflow.append((e, idx[CAPACITY:], w_e[e][CAPACITY:]))
            idx = idx[:CAPACITY]
            idx_e[e] = idx
            w_e[e] = w_e[e][:CAPACITY]
        # xs[cb, p, dc, j] = x[token cb*512+j, d dc*128+p], zero-padded
        xt = np.zeros((CAPACITY, D_MODEL), dtype=BF16)
        xt[:len(idx)] = x2_bf[idx]
        xs = np.ascontiguousarray(
            xt.reshape(nblk, 512, DC, P).transpose(0, 3, 2, 1)
        )
        # w1s{i}[p, dc, h'] = W1[e, dc*128+p, c0+h']
        w1r = W1[e].astype(BF16).reshape(DC, P, FFN_HIDDEN)
        # w2d[dt, p, hc, d'] = W2[e, hc*128+p, dt*128+d']
        w2sl = np.ascontiguousarray(
            W2[e].astype(BF16).reshape(HC, P, DT, P).transpose(2, 1, 0, 3))
        bias = np.concatenate(
            [b1[e].reshape(HC, P).T, b2[e].reshape(DC, P).T], axis=1)
        m = {"xs": xs, "w2d": w2sl, "bias": np.ascontiguousarray(bias)}
        c0 = 0
        for i, w in enumerate(W1_SLICES):
            m[f"w1s{i}"] = np.ascontiguousarray(
                w1r[:, :, c0:c0 + w].transpose(1, 0, 2))
            c0 += w
        in_maps.append(m)

    nc = _get_nc()
    res = None
    for attempt in range(3):  # transient NRT device errors: retry
        try:
            res = run_bass_kernel_spmd(
                nc, in_maps, core_ids=list(range(N_EXPERTS)), trace=TRACE
            )
            break
        except Exception:
            if attempt == 2:
                raise
            import time
            time.sleep(2.0)
    LAST_EXEC_NS = res.exec_time_ns
    if res.instructions_and_trace is not None:
        LAST_TRACE_PATH = res.instructions_and_trace[1]

    out = np.zeros((T, D), dtype=np.float32)
    for e in range(N_EXPERTS):
        idx = idx_e[e]
        if len(idx) == 0:
            continue
        # yt[cb, p, dt, j] -> y[token cb*512+j, d dt*128+p]
        ye = res.results[e]["yt"].transpose(0, 3, 2, 1).reshape(
            CAPACITY, D_MODEL).astype(np.float32)
        out[idx] += w_e[e][:, None] * ye[:len(idx)]
    for e, idx, w in overflow:
        ye = _host_ffn_f64(x2[idx], W1[e], b1[e], W2[e], b2[e])
        out[idx] += (w[:, None] * ye).astype(np.float32)

    return out.reshape(B, S, D)


# revision 17
# speedup vs baseline: 1.2029x; 1.0013x over previous
"""MoE (top-2 of 8 experts, D=1024, FFN=4096) on 8 Trainium2 NeuronCores.

Strategy (expert-parallel, per the sharding hint):
  - Host computes the gating softmax + top-2 routing (this IS the sharding
    step: it decides which tokens go to which core).
  - Core e holds expert e's weights (bf16) and runs the FFN
    y = gelu(x @ W1 + b1) @ W2 + b2 for the tokens routed to expert e,
    capacity-padded to C tokens, activations streamed as [D, C] so the
    contraction dim always sits on SBUF partitions (no transposes on device).
  - Host scatter-adds the combine-weighted expert outputs back into the
    full [B, S, D] output.

All matmuls run in bf16 with fp32 PSUM accumulation; bias+gelu epilogues on
the scalar engine read PSUM directly.

Perf notes (from NTFF trace analysis; ~463us total vs 470us baseline):
  - The 2048 matmuls/core stream at the bf16 PE floor (215.9 ns per
    [128x128]@[128,512], zero inter-matmul gaps); the recoverable time was
    startup, tail, and the HAM cold-clock penalty.
  - All DRAM tensors are host-pre-transposed into partition-major tile
    layouts so each DMA descriptor is one contiguous 2-8KB line per
    partition (the naive [D, C] views produced 512B lines and ~4x lower
    DMA throughput).
  - Queue facts: sync and scalar are hardware-DGE (fast); gpsimd is
    software-DGE (~4us late first packet, ~3x slower) and only carries
    slack traffic (bias, x prefetches, mid-kernel y writes). Completion
    semaphores fire in per-queue FIFO order, so the first matmul's gating
    pair (x block 0, W1 slice 0) leads the two fast queues with nothing
    in front; measured startup floor is ~15.4us.
  - W1 streams in column slices sized so each slice lands ahead of the
    matmul chain that consumes it; W2 follows in the 8 d-tile slices
    GEMM2 consumes.
  - A burst of scratch matmuls bridges the DMA wait after the start
    barrier so the HAM activity monitor never sees an idle window and the
    real matmuls run at 2.4 GHz from the first instruction.
  - y is written back in bf16 (error contribution ~0.07% RMS), streamed
    per d-tile during GEMM2; the very last write is split across the two
    fast queues so the end-of-kernel drain is ~1.5us.
"""

import math

import numpy as np
import ml_dtypes

D_MODEL = 1024
FFN_HIDDEN = 4096
N_EXPERTS = 8
TOP_K = 2
CAPACITY = 2048          # per-expert token capacity (mean load 2048); the rare
                         # overflow tokens take the exact host fallback path
CBLKS = [512, 512, 512, 512]        # tokens per matmul moving-operand block
assert sum(CBLKS) == CAPACITY
P = 128                  # SBUF partitions
DC = D_MODEL // P        # 8 d-chunks of 128
HC = FFN_HIDDEN // P     # 32 h-chunks of 128
DT = D_MODEL // P        # 8 d-tiles of GEMM2 output
W1_SLICES = [256, 256, 256, 256, 512, 512, 512, 512, 512, 512]
assert sum(W1_SLICES) == FFN_HIDDEN  # small early slices = short critical path
N_WARM = 30              # scratch matmuls to warm the PE clock gate

BF16 = ml_dtypes.bfloat16

_ACT_FUNC = "Gelu"       # sim_check overrides to "Tanh" (CoreSim lacks Gelu)
TRACE = False            # test harness sets True to collect an NTFF profile
LAST_EXEC_NS = None
LAST_TRACE_PATH = None

_NC_CACHE = {}


def _build_bass():
    import concourse.bacc as bacc
    import concourse.mybir as mybir
    import concourse.tile as tile

    nc = bacc.Bacc("TRN2", target_bir_lowering=False, debug=False)
    dt = mybir.dt

    # All tensors arrive host-pre-transposed into the exact SBUF tile
    # layouts (partition dim first) so each DMA descriptor is one
    # contiguous 2-8KB line per partition.
    xs = nc.dram_tensor("xs", [len(CBLKS), P, DC, 512], dt.bfloat16,
                        kind="ExternalInput")
    w1s = [nc.dram_tensor(f"w1s{i}", [P, DC, w], dt.bfloat16,
                          kind="ExternalInput")
           for i, w in enumerate(W1_SLICES)]
    w2d = nc.dram_tensor("w2d", [DT, P, HC, P], dt.bfloat16,
                         kind="ExternalInput")
    bias = nc.dram_tensor("bias", [P, HC + DC], dt.float32,
                          kind="ExternalInput")
    yt = nc.dram_tensor("yt", [len(CBLKS), P, DT, 512], dt.bfloat16,
                        kind="ExternalOutput")

    gelu = getattr(mybir.ActivationFunctionType, _ACT_FUNC)
    ident = mybir.ActivationFunctionType.Identity

    with tile.TileContext(nc) as tc:
        with (
            tc.tile_pool(name="warm", bufs=1) as warmpool,
            tc.tile_pool(name="wpool", bufs=1) as wpool,
            tc.tile_pool(name="bpool", bufs=1) as bpool,
            tc.tile_pool(name="xpool", bufs=1) as xpool,
            tc.tile_pool(name="hpool", bufs=1) as hpool,
            tc.tile_pool(name="ypool", bufs=4) as ypool,
            tc.tile_pool(name="ps1", bufs=4, space="PSUM") as ps1pool,
            tc.tile_pool(name="ps2", bufs=4, space="PSUM") as ps2pool,
        ):
            # --- critical startup DMAs ------------------------------------
            # Queue facts (measured): sync and scalar are hardware-DGE
            # (fast, ~0.5MB/1.2us once streaming); gpsimd is software-DGE
            # (first packet ~4us late, ~3x slower) -- never put critical
            # data there. Completion sems fire in per-queue FIFO order, so
            # the first matmul's gating pair leads both fast queues: x0's
            # two halves go sync (ahead of W1) + scalar (behind its
            # activation-table load); W1 slices follow on sync. The bias
            # leads the slow gpsimd queue: it lands ~15us, which only
            # delays the first gelu epilogue (ps1 has 4 banks of slack),
            # never the PE.
            x0 = xpool.tile([P, DC, 512], dt.bfloat16, tag="x")
            hh = DC // 2
            nc.sync.dma_start(x0[:, 0:hh, :], xs[0, :, 0:hh, :])
            nc.scalar.dma_start(x0[:, hh:DC, :], xs[0, :, hh:DC, :])

            b_sb = bpool.tile([P, HC + DC], dt.float32, tag="bias")
            nc.gpsimd.dma_start(b_sb[:], bias[:, :])

            w1_sb = []   # (col0, width, tile)
            col = 0
            for si, w in enumerate(W1_SLICES):
                t = wpool.tile([P, DC, w], dt.bfloat16, tag=f"w1_{si}")
                nc.sync.dma_start(t[:], w1s[si][:])
                w1_sb.append((col, w, t))
                col += w

            # --- PE clock-gate warmup ------------------------------------
            # The PE powers up throttled to 1.2 GHz and reaches 2.4 GHz after
            # ~3.4us of sustained activity; bridge the whole DMA wait
            # (~8.9us -> ~15us) with scratch matmuls so the real ones run
            # at full clock and the activity monitor never sees an idle gap.
            warm_sb = warmpool.tile([P, 256], dt.bfloat16, tag="warm")
            nc.vector.memset(warm_sb[:], 0)
            warm_ps = ps1pool.tile([P, 256], dt.float32, tag="ps1")
            for _ in range(N_WARM):
                nc.tensor.matmul(
                    warm_ps[:], warm_sb[:, 0:P], warm_sb[:], start=True,
                    stop=True,
                )

            def load_x_block(cb):      # prefetch on the gpsimd queue
                t = xpool.tile([P, DC, 512], dt.bfloat16, tag="x")
                nc.gpsimd.dma_start(t[:], xs[cb])
                return t

            def w1_tile(hc, dc):
                """lhsT slice [P, 128] for h-tile hc, d-chunk dc."""
                h0 = hc * P
                for col0, w, t in w1_sb:
                    if col0 <= h0 < col0 + w:
                        return t[:, dc, h0 - col0:h0 - col0 + P]
                raise AssertionError(hc)

            x_tiles = [x0]
            w2_sb = None

            for cb, cblk in enumerate(CBLKS):
                x_t = x_tiles[cb]
                if cb + 1 < len(CBLKS):  # prefetch next activation block
                    x_tiles.append(load_x_block(cb + 1))

                h_t = hpool.tile([P, HC, cblk], dt.bfloat16, tag="h")

                # GEMM1: H1^T[h, c] = sum_d W1[d, h] * X^T[d, c]
                for hc in range(HC):
                    ps = ps1pool.tile([P, cblk], dt.float32, tag="ps1")
                    for dc in range(DC):
                        nc.tensor.matmul(
                            ps[:],
                            w1_tile(hc, dc),
                            x_t[:, dc, :],
                            start=(dc == 0),
                            stop=(dc == DC - 1),
                        )
                    nc.scalar.activation(
                        h_t[:, hc, :], ps[:], gelu, bias=b_sb[:, hc:hc + 1]
                    )

                if w2_sb is None:  # W2 streams in behind W1, before GEMM2 use
                    w2_sb = []
                    for g in range(DT):  # 8 d-tile slices, consumption order
                        t = wpool.tile([P, HC, P], dt.bfloat16, tag=f"w2_{g}")
                        nc.sync.dma_start(t[:], w2d[g])
                        w2_sb.append(t)

                # GEMM2: Y^T[d, c] = sum_h W2[h, d] * H1^T[h, c]
                for dt_i in range(DT):
                    ps2 = ps2pool.tile([P, cblk], dt.float32, tag="ps2")
                    for hc in range(HC):
                        nc.tensor.matmul(
                            ps2[:],
                            w2_sb[dt_i][:, hc, :],
                            h_t[:, hc, :],
                            start=(hc == 0),
                            stop=(hc == HC - 1),
                        )
                    y_t = ypool.tile([P, cblk], dt.bfloat16, tag="y")
                    nc.scalar.activation(
                        y_t[:], ps2[:], ident,
                        bias=b_sb[:, HC + dt_i:HC + dt_i + 1]
                    )
                    # stream y out during GEMM2, alternating queues; the
                    # very last write is split across two queues so the
                    # end-of-kernel drain generates descriptors in parallel
                    last = (cb == len(CBLKS) - 1) and (dt_i == DT - 1)
                    if last:
                        # both fast HWDGE queues (sync is idle by now)
                        nc.sync.dma_start(yt[cb, 0:P // 2, dt_i, :],
                                          y_t[0:P // 2])
                        nc.scalar.dma_start(yt[cb, P // 2:P, dt_i, :],
                                            y_t[P // 2:P])
                    elif dt_i % 2 == 0:
                        # last block's writes avoid the slow software DGE so
                        # nothing drags into the end-of-kernel drain
                        eng = nc.sync if cb == len(CBLKS) - 1 else nc.gpsimd
                        eng.dma_start(yt[cb, :, dt_i, :], y_t[:])
                    else:
                        nc.scalar.dma_start(yt[cb, :, dt_i, :], y_t[:])

    nc.compile()
    return nc


def _get_nc():
    if "nc" not in _NC_CACHE:
        _NC_CACHE["nc"] = _build_bass()
    return _NC_CACHE["nc"]


def _route(x2, w_gate):
    """fp32 gating softmax + distinct top-2, matching the reference."""
    T = x2.shape[0]
    logits = x2 @ w_gate.T                      # [T, E] fp32
    m = logits.max(1, keepdims=True)
    e = np.exp(logits - m, dtype=np.float32)
    p = e / e.sum(1, keepdims=True)
    i1 = p.argmax(1)
    pm = p.copy()
    pm[np.arange(T), i1] = -1.0
    i2 = pm.argmax(1)
    s1 = p[np.arange(T), i1]
    s2 = p[np.arange(T), i2]
    return i1, i2, s1, s2


def _host_ffn_f64(xrows, W1e, b1e, W2e, b2e):
    """Exact-math fallback FFN for capacity-overflow tokens (rare)."""
    h = xrows.astype(np.float64) @ W1e.astype(np.float64) + b1e.astype(np.float64)
    try:
        from scipy.special import erf
        g = 0.5 * h * (1.0 + erf(h / math.sqrt(2.0)))
    except ImportError:
        g = 0.5 * h * (1.0 + np.frompyfunc(math.erf, 1, 1)(h / math.sqrt(2.0)).astype(np.float64))
    return g @ W2e.astype(np.float64) + b2e.astype(np.float64)


def _ensure_ntff_hook():
    """Register the axon NTFF profile hook if the image's antenv lacks it.

    Only used on TRACE=True (dev profiling) runs; never on the plain path.
    """
    import sys
    import types
    try:
        import antenv.axon_hooks  # noqa: F401
        return
    except ImportError:
        pass
    hook = None
    try:
        from trn_agent_boot.trn_boot import _ntff_profile_via_ctypes
        hook = _ntff_profile_via_ctypes("/opt/axon/libaxon_pjrt.so")
    except Exception:
        hook = None
    mod = types.ModuleType("antenv.axon_hooks")
    mod.get_axon_ntff_profile_hook = lambda: hook
    mod.set_axon_ntff_profile_hook = lambda h: None
    sys.modules["antenv.axon_hooks"] = mod
    try:
        import antenv
        antenv.axon_hooks = mod
    except Exception:
        pass


def kernel(x, w_gate, W1, b1, W2, b2):
    global LAST_EXEC_NS, LAST_TRACE_PATH
    from concourse.bass_utils import run_bass_kernel_spmd
    if TRACE:
        _ensure_ntff_hook()

    x = np.asarray(x, dtype=np.float32)
    w_gate = np.asarray(w_gate, dtype=np.float32)
    W1 = np.asarray(W1, dtype=np.float32)
    b1 = np.asarray(b1, dtype=np.float32)
    W2 = np.asarray(W2, dtype=np.float32)
    b2 = np.asarray(b2, dtype=np.float32)

    B, S, D = x.shape
    T = B * S
    x2 = np.ascontiguousarray(x.reshape(T, D))

    i1, i2, s1, s2 = _route(x2, w_gate)

    # Per-expert dispatch lists (a token appears at most once per expert).
    idx_e, w_e = [], []
    for e in range(N_EXPERTS):
        a = np.nonzero(i1 == e)[0]
        b = np.nonzero(i2 == e)[0]
        idx = np.concatenate([a, b])
        w = np.concatenate([s1[a], s2[b]]).astype(np.float32)
        idx_e.append(idx)
        w_e.append(w)

    x2_bf = x2.astype(BF16)
    nblk = len(CBLKS)
    in_maps = []
    overflow = []  # (expert, token_ids, weights) beyond capacity
    for e in range(N_EXPERTS):
        idx = idx_e[e]
        if len(idx) > CAPACITY:
            overflow.append((e, idx[CAPACITY:], w_e[e][CAPACITY:]))
            idx = idx[:CAPACITY]
            idx_e[e] = idx
            w_e[e] = w_e[e][:CAPACITY]
        # xs[cb, p, dc, j] = x[token cb*512+j, d dc*128+p], zero-padded
        xt = np.zeros((CAPACITY, D_MODEL), dtype=BF16)
        xt[:len(idx)] = x2_bf[idx]
        xs = np.ascontiguousarray(
            xt.reshape(nblk, 512, DC, P).transpose(0, 3, 2, 1)
        )
        # w1s{i}[p, dc, h'] = W1[e, dc*128+p, c0+h']
        w1r = W1[e].astype(BF16).reshape(DC, P, FFN_HIDDEN)
        # w2d[dt, p, hc, d'] = W2[e, hc*128+p, dt*128+d']
        w2sl = np.ascontiguousarray(
            W2[e].astype(BF16).reshape(HC, P, DT, P).transpose(2, 1, 0, 3))
        bias = np.concatenate(
            [b1[e].reshape(HC, P).T, b2[e].reshape(DC, P).T], axis=1)
        m = {"xs": xs, "w2d": w2sl, "bias": np.ascontiguousarray(bias)}
        c0 = 0
        for i, w in enumerate(W1_SLICES):
            m[f"w1s{i}"] = np.ascontiguousarray(
                w1r[:, :, c0:c0 + w].transpose(1, 0, 2))
            c0 += w
        in_maps.append(m)

    nc = _get_nc()
    res = None
    for attempt in range(3):  # transient NRT device errors: retry
        try:
            res = run_bass_kernel_spmd(
                nc, in_maps, core_ids=list(range(N_EXPERTS)), trace=TRACE
            )
            break
        except Exception:
            if attempt == 2:
                raise
            import time
            time.sleep(2.0)
    LAST_EXEC_NS = res.exec_time_ns
    if res.instructions_and_trace is not None:
        LAST_TRACE_PATH = res.instructions_and_trace[1]

    out = np.zeros((T, D), dtype=np.float32)
    for e in range(N_EXPERTS):
        idx = idx_e[e]
        if len(idx) == 0:
            continue
        # yt[cb, p, dt, j] -> y[token cb*512+j, d dt*128+p]
        ye = res.results[e]["yt"].transpose(0, 3, 2, 1).reshape(
            CAPACITY, D_MODEL).astype(np.float32)
        out[idx] += w_e[e][:, None] * ye[:len(idx)]
    for e, idx, w in overflow:
        ye = _host_ffn_f64(x2[idx], W1[e], b1[e], W2[e], b2[e])
        out[idx] += (w[:, None] * ye).astype(np.float32)

    return out.reshape(B, S, D)
